# revision 1
# baseline (speedup 1.0000x reference)
"""CrossAttention kernel for 8 Trainium2 NeuronCores.

Sharding: data-parallel over batch B=16 -> 2 batches per core across 8
cores (the k/v unfold, per-channel attention and fold are batch-local;
only the small 1x1-conv weights / pos-bias tables are replicated).

Per-shard computation (all shapes hardcoded per the problem spec):
  q:  [2, 64, 32, 32]    lf: [2, 64, 256, 256]
  1x1 convs (Wq,Wk,Wv), unfold into 8x8 grid of 32x32 patches,
  per-channel 64x64 patch attention over L=1024 pixels, fold,
  output 1x1 conv (Wo).
"""

import numpy as np

PH = PW = 8
PN = PH * PW            # 64
B, QD, KVD, H, W = 16, 64, 64, 256, 256
KH, KW = H // PH, W // PW   # 32, 32
L = KH * KW             # 1024
NCORES = 8
BPC = B // NCORES       # 2 batches per core

_COMPILED = {}


def _shard_fn(q, lf, Wq, bq, Wk, bk, Wv, bv, abs_k, abs_v, Wo, bo):
    """Per-device computation on a [BPC, ...] batch shard."""
    import jax.numpy as jnp

    def conv1x1(x, w, b):
        # x: [b,C,H,W], w: [O,C]
        return jnp.einsum('bchw,oc->bohw', x, w) + b[None, :, None, None]

    def unfold(x, pos):
        # x: [b, PN, H, W] -> [b, C, PN(patch), kh, kw] + pos bias
        bb, C, hh, ww = x.shape
        x = x.reshape(bb, C, PH, KH, PW, KW).transpose(0, 1, 3, 5, 2, 4)
        x = x.reshape(bb, C, KH, KW, PN).transpose(0, 1, 4, 2, 3)
        return x + pos[None, :, :, None, None]

    qp = conv1x1(q, Wq, bq).reshape(BPC, PN, L)
    k = unfold(conv1x1(lf, Wk, bk), abs_k).reshape(BPC, PN, PN, L)
    v = unfold(conv1x1(lf, Wv, bv), abs_v).reshape(BPC, PN, PN, L)
    attn = jnp.einsum('bpl,biql->bipq', qp, k)
    attn = jax.nn.softmax(attn, axis=-1)
    out = jnp.einsum('bipq,biql->bipl', attn, v)
    out = out.reshape(BPC, PN, PH, PW, KH, KW).transpose(0, 1, 2, 4, 3, 5)
    out = out.reshape(BPC, PN, H, W)
    return conv1x1(out, Wo, bo)


import jax  # noqa: E402  (needed inside _shard_fn for jax.nn.softmax)


def _get_pmapped():
    if 'fn' not in _COMPILED:
        # weights are replicated (in_axes=None), activations batch-sharded
        _COMPILED['fn'] = jax.pmap(
            _shard_fn,
            axis_name='cores',
            in_axes=(0, 0) + (None,) * 10,
            devices=jax.devices()[:NCORES],
        )
    return _COMPILED['fn']


def kernel(q, lf, Wq, bq, Wk, bk, Wv, bv, abs_k, abs_v, Wo, bo):
    q = np.asarray(q, np.float32).reshape(NCORES, BPC, QD, KH, KW)
    lf = np.asarray(lf, np.float32).reshape(NCORES, BPC, KVD, H, W)
    args = [np.asarray(a, np.float32)
            for a in (Wq, bq, Wk, bk, Wv, bv, abs_k, abs_v, Wo, bo)]
    fn = _get_pmapped()
    out = fn(q, lf, *args)                    # [8, 2, 64, 256, 256]
    return np.asarray(out, np.float32).reshape(B, KVD, H, W)


if __name__ == '__main__':
    rng = np.random.default_rng(0)
    ins = {
        'q': rng.standard_normal((B, QD, KH, KW), np.float32),
        'lf': rng.standard_normal((B, KVD, H, W), np.float32),
        'Wq': rng.standard_normal((PN, QD), np.float32) * 0.02,
        'bq': np.zeros(PN, np.float32),
        'Wk': rng.standard_normal((PN, KVD), np.float32) * 0.02,
        'bk': np.zeros(PN, np.float32),
        'Wv': rng.standard_normal((PN, KVD), np.float32) * 0.02,
        'bv': np.zeros(PN, np.float32),
        'abs_k': rng.standard_normal((PN, PN), np.float32) * 0.02,
        'abs_v': rng.standard_normal((PN, PN), np.float32) * 0.02,
        'Wo': rng.standard_normal((KVD, PN), np.float32) * 0.02,
        'bo': np.zeros(KVD, np.float32),
    }
    out = kernel(**ins)
    print(out.shape, out.dtype, float(np.abs(out).mean()))
